# revision 1
# baseline (speedup 1.0000x reference)
"""Trainium2 Bass kernel for per-sample channel attention.

Reference computation (per sample n of 32):
    e  = x[n].reshape(C, HW)                      # C=512, HW=1024
    q  = sigmoid(relu(e @ wq1) @ wq2)             # [C, HW]
    k  = sigmoid(relu(e @ wk1) @ wk2)             # [C, HW]
    v  = sigmoid(relu(e @ wv1) @ wv2)             # [C, HW]
    s  = q @ k.T / sqrt(C)                        # [C, C]
    o  = softmax(s, axis=-1) @ v                  # [C, HW]

Strategy: data-parallel over batch N across 8 cores (4 samples each),
weights replicated. All matmuls are arranged so no on-device transpose is
ever needed:
  - host passes xT = x transposed to [n, HW, C]
  - layer1 produces h^T = [HID, C], layer2 produces q^T/k^T = [HW, C] and
    v = [C, HW] (natural)
  - scores are computed TRANSPOSED: s^T = (k^T).T @ q^T  -> [d, c] with the
    softmax axis d on partitions
  - softmax needs no max subtraction (q,k in (0,1) bound s <= sqrt(C)*~45,
    exp() cannot overflow fp32); denominator Z[c] = sum_d exp(s^T)[d,c] is a
    ones-vector matmul; o = (E.T @ v) * (1/Z) per-partition broadcast.

Matmul operands are bf16 (fp32 PSUM accumulation); measured end-to-end
absmax error vs the fp32 reference is ~1.0e-3 on output scale ~0.9-1.0.
Measured HW exec time: ~164 us per core median (best 162; run-to-run
+-1.5 us device noise). Breakdown: 640 N=512 matmuls at the 216 ns issue
floor (~138 us, zero mid-stream PE gaps in the trace) + 64 cheap N=1
softmax-denominator matmuls + ~7 us fixed Tile preamble + ~6 us
DMA-bound startup (hidden behind warm-up + arrival-ordered layer-1
accumulation) + ~10 us fixed teardown barrier.
"""

import math

import numpy as np
import ml_dtypes

N, C, H, W, R = 32, 512, 32, 32, 4
HW = H * W            # 1024
HID = HW // R         # 256
NCORES = 8
PER = N // NCORES     # samples per core
P = 128               # partitions

KO1 = HW // P         # 8  k-chunks for layer1 / scores contraction
KO2 = HID // P        # 2  k-chunks for layer2
MT_H = HID // P       # 2  m-tiles of h^T
MT_E = HW // P        # 8  m-tiles of q^T/k^T (HW rows)
MT_C = C // P         # 4  m-tiles over C
NH = HW // C          # 2  halves of HW free dim (512 each)

# matmul operand dtype: "bf16" | "fp32r" | "fp32"
MM_DT = "bf16"

_STATE = {}


def _np_dt():
    return ml_dtypes.bfloat16 if MM_DT == "bf16" else np.float32


def _build_nc():
    import concourse.bass as bass  # noqa: F401
    import concourse.mybir as mybir
    import concourse.tile as tile
    from concourse import bacc

    dt = mybir.dt.bfloat16 if MM_DT == "bf16" else mybir.dt.float32
    f32 = mybir.dt.float32
    # dtype used on the matmul APs (bitcast for fp32r)
    mm_cast = mybir.dt.float32r if MM_DT == "fp32r" else None

    def mmap(ap):
        return ap.bitcast(mm_cast) if mm_cast is not None else ap

    nc = bacc.Bacc("TRN2")

    # All inputs arrive pre-swizzled to SBUF-native layout [P, o, m]
    # (partition-major, contiguous 8KB runs per partition) for fast DMA.
    xt = nc.dram_tensor("xt", [PER, P, KO1, C], dt, kind="ExternalInput")
    ws1 = {
        kind: nc.dram_tensor(
            f"w{kind}1", [P, KO1, HID], dt, kind="ExternalInput"
        )
        for kind in "qkv"
    }
    ws2 = {
        kind: nc.dram_tensor(
            f"w{kind}2", [P, KO2, HW], dt, kind="ExternalInput"
        )
        for kind in "qkv"
    }
    out = nc.dram_tensor("o", [PER, C, HW], f32, kind="ExternalOutput")

    inv_sqrt_c = 1.0 / math.sqrt(C)

    with tile.TileContext(nc) as tc:
        with (
            tc.tile_pool(name="singles", bufs=1) as singles,
            tc.tile_pool(name="acts", bufs=2) as acts,
            tc.tile_pool(name="hts", bufs=3) as hts,
            tc.tile_pool(name="obuf", bufs=3) as obuf,
            tc.tile_pool(name="psum", bufs=6, space="PSUM") as psum,
            tc.tile_pool(name="psz", bufs=2, space="PSUM") as psz,
        ):
            # Weights, loaded once, K on partitions (natural [in, out]
            # layout). DMA order matters for startup latency: the first
            # matmuls need only wq1 + eT(s0), so issue those two first and
            # the remaining weights behind them.
            w1_sb = {}
            w2_sb = {}
            for kind in "qkv":
                w1_sb[kind] = singles.tile(
                    [P, KO1, HID], dt, tag=f"w1{kind}", name=f"w1{kind}"
                )
                w2_sb[kind] = singles.tile(
                    [P, KO2, HW], dt, tag=f"w2{kind}", name=f"w2{kind}"
                )

            # Input DMAs in need-order on the sync HWDGE queue: wq1,
            # eT(s0), wq2, then the k/v weights. Inputs are pre-swizzled on
            # the host so each partition reads one contiguous run.
            warm_sb = singles.tile([P, C], dt, tag="warm", name="warm")
            nc.gpsimd.memset(warm_sb, 0.0)

            # HWDGE (sync) sustains ~180 GB/s, SWDGE (gpsimd) ~80 GB/s and
            # they run in parallel; split the critical first wave so both
            # finish together: wq1 + 5 eT chunks on sync, 3 eT chunks on
            # gpsimd.
            nc.sync.dma_start(out=w1_sb["q"], in_=ws1["q"][:])
            eT0 = acts.tile([P, KO1, C], dt, tag="eT", name="eT")
            nc.gpsimd.dma_start(out=eT0[:, 6:, :], in_=xt[0][:, 6:, :])
            nc.sync.dma_start(out=eT0[:, :2, :], in_=xt[0][:, :2, :])
            nc.sync.dma_start(out=eT0[:, 2:4, :], in_=xt[0][:, 2:4, :])
            nc.sync.dma_start(out=eT0[:, 4:6, :], in_=xt[0][:, 4:6, :])
            nc.sync.dma_start(out=w2_sb["q"], in_=ws2["q"][:])
            nc.sync.dma_start(out=w1_sb["k"], in_=ws1["k"][:])
            nc.sync.dma_start(out=w2_sb["k"], in_=ws2["k"][:])
            nc.sync.dma_start(out=w1_sb["v"], in_=ws1["v"][:])
            nc.sync.dma_start(out=w2_sb["v"], in_=ws2["v"][:])

            ones_sb = singles.tile([P, 1], dt, tag="ones", name="ones")
            nc.vector.memset(ones_sb, 1.0)
            actw = singles.tile([P, 1], f32, tag="actw", name="actw")

            # PE clock warm-up: the HAM un-throttles (1.2 -> 2.4 GHz) only
            # after ~3.4us of sustained matmul activity. Fill the initial
            # DMA-wait window with dummy matmuls on zeros so the real
            # matmuls start closer to full clock.
            warm_ps = psum.tile([P, C], f32, tag="ps", name="ps")
            for _ in range(10):
                nc.tensor.matmul(
                    warm_ps, warm_sb[:, :P], warm_sb, start=True, stop=True
                )

            for s in range(PER):
                if s == 0:
                    eT = eT0
                else:
                    eT = acts.tile([P, KO1, C], dt, tag="eT", name="eT")
                    nc.scalar.dma_start(out=eT, in_=xt[s])

                qT = acts.tile([P, MT_E, C], dt, tag="qT", name="qT")
                kT = acts.tile([P, MT_E, C], dt, tag="kT", name="kT")
                v = acts.tile([P, MT_C, HW], dt, tag="v", name="v")

                # sample 0's eT chunks arrive gpsimd(5,6,7) then sync(0,1),
                # (2,3,4); accumulate in arrival order (order is free)
                korder = (6, 7, 0, 1, 2, 3, 4, 5) if s == 0 else tuple(range(KO1))
                for kind in "qkv":
                    # layer 1: h^T[r, c] accumulated over HW chunks
                    hT = hts.tile([P, KO2, C], dt, tag="hT", name="hT")
                    for m in range(MT_H):
                        ps = psum.tile([P, C], f32, tag="ps", name="ps")
                        for ki, k in enumerate(korder):
                            nc.tensor.matmul(
                                ps,
                                mmap(w1_sb[kind][:, k, m * P : (m + 1) * P]),
                                mmap(eT[:, k, :]),
                                start=(ki == 0),
                                stop=(ki == KO1 - 1),
                            )
                        # relu PSUM -> SBUF (cast to mm dtype)
                        nc.vector.tensor_scalar_max(hT[:, m, :], ps, 0.0)

                    if kind in ("q", "k"):
                        dst = qT if kind == "q" else kT
                        # layer 2 transposed: q^T[e, c] = w2.T-free @ h^T
                        for m in range(MT_E):
                            ps = psum.tile([P, C], f32, tag="ps", name="ps")
                            for k in range(KO2):
                                nc.tensor.matmul(
                                    ps,
                                    mmap(w2_sb[kind][:, k, m * P : (m + 1) * P]),
                                    mmap(hT[:, k, :]),
                                    start=(k == 0),
                                    stop=(k == KO2 - 1),
                                )
                            nc.scalar.activation(
                                dst[:, m, :], ps,
                                mybir.ActivationFunctionType.Sigmoid,
                            )
                    else:
                        # v natural: v[d, e] = (h^T).T @ w2
                        for m in range(MT_C):
                            for nh in range(NH):
                                ps = psum.tile([P, C], f32, tag="ps", name="ps")
                                for k in range(KO2):
                                    nc.tensor.matmul(
                                        ps,
                                        mmap(hT[:, k, m * P : (m + 1) * P]),
                                        mmap(
                                            w2_sb[kind][:, k, nh * C : (nh + 1) * C]
                                        ),
                                        start=(k == 0),
                                        stop=(k == KO2 - 1),
                                    )
                                nc.scalar.activation(
                                    v[:, m, nh * C : (nh + 1) * C], ps,
                                    mybir.ActivationFunctionType.Sigmoid,
                                )
                    # pull the exp LUT reload forward (depends on last
                    # sigmoid output so it lands after all sigmoids)
                    nc.scalar.activation(
                        actw, v[:, MT_C - 1, HW - 1 :],
                        mybir.ActivationFunctionType.Exp,
                    )

                # scores transposed: s^T[d, c] = (k^T).T @ q^T, then
                # E = exp(s^T / sqrt(C)) fused into the activation scale.
                E = acts.tile([P, MT_C, C], dt, tag="E", name="E")
                for m in range(MT_C):
                    ps = psum.tile([P, C], f32, tag="ps", name="ps")
                    for k in range(MT_E):
                        nc.tensor.matmul(
                            ps,
                            mmap(kT[:, k, m * P : (m + 1) * P]),
                            mmap(qT[:, k, :]),
                            start=(k == 0),
                            stop=(k == MT_E - 1),
                        )
                    nc.scalar.activation(
                        E[:, m, :], ps,
                        mybir.ActivationFunctionType.Exp,
                        scale=inv_sqrt_c,
                    )
                if s < PER - 1:
                    # pull the sigmoid LUT reload into this slack window
                    # (depends on the last exp so it cannot reorder earlier)
                    nc.scalar.activation(
                        actw, E[:, MT_C - 1, :1],
                        mybir.ActivationFunctionType.Sigmoid,
                    )

                # softmax denominator: Z[c] = sum_d E[d, c]  (ones matmul),
                # then reciprocal.
                rz = obuf.tile([P, MT_C], f32, tag="rz", name="rz")
                for m in range(MT_C):
                    pz = psz.tile([P, 1], f32, tag="pz", name="pz")
                    for k in range(MT_C):
                        nc.tensor.matmul(
                            pz,
                            mmap(E[:, k, m * P : (m + 1) * P]),
                            mmap(ones_sb),
                            start=(k == 0),
                            stop=(k == MT_C - 1),
                        )
                    nc.vector.reciprocal(rz[:, m : m + 1], pz)

                # o[c, e] = (E.T @ v) * (1/Z)[c]
                out_r = out[s].rearrange("(mo p) e -> p mo e", p=P)
                for m in range(MT_C):
                    ob = obuf.tile([P, HW], f32, tag="ob", name="ob")
                    for nh in range(NH):
                        oeng = (
                            (nc.sync, nc.scalar)[(m * NH + nh) % 2]
                            if s == PER - 1
                            else nc.sync
                        )
                        # the very last output group is split into two N=256
                        # halves so the tail chain (mul -> store -> drain)
                        # after the final matmul is half as long
                        last = s == PER - 1 and m == MT_C - 1 and nh == NH - 1
                        for h in range(2 if last else 1):
                            nw = C // 2 if last else C
                            base = nh * C + h * nw
                            ps = psum.tile([P, C], f32, tag="ps", name="ps")
                            for k in range(MT_C):
                                nc.tensor.matmul(
                                    ps[:, :nw],
                                    mmap(E[:, k, m * P : (m + 1) * P]),
                                    mmap(v[:, k, base : base + nw]),
                                    start=(k == 0),
                                    stop=(k == MT_C - 1),
                                )
                            nc.vector.tensor_scalar_mul(
                                ob[:, base : base + nw],
                                ps[:, :nw],
                                rz[:, m : m + 1],
                            )
                            oeng.dma_start(
                                out=out_r[:, m, base : base + nw],
                                in_=ob[:, base : base + nw],
                            )

    nc.finalize()
    return nc


def _get_nc():
    key = ("nc", MM_DT)
    if key not in _STATE:
        _STATE[key] = _build_nc()
    return _STATE[key]


def kernel(**inputs):
    x = np.asarray(inputs["x"])
    np_dt = _np_dt()

    # host-side reformat to SBUF-native layouts (+ dtype cast):
    #   x:  [N, C, H, W] -> e^T [N, HW, C] -> [N, P, KO1, C]
    #   w1: [HW, HID]    -> [P, KO1, HID];  w2: [HID, HW] -> [P, KO2, HW]
    xt = np.ascontiguousarray(
        x.reshape(N, C, HW)
        .transpose(0, 2, 1)
        .reshape(N, KO1, P, C)
        .transpose(0, 2, 1, 3)
    ).astype(np_dt)
    w = {}
    for name in ("wq1", "wk1", "wv1"):
        a = np.asarray(inputs[name]).astype(np_dt)
        w[name] = np.ascontiguousarray(
            a.reshape(KO1, P, HID).transpose(1, 0, 2)
        )
    for name in ("wq2", "wk2", "wv2"):
        a = np.asarray(inputs[name]).astype(np_dt)
        w[name] = np.ascontiguousarray(
            a.reshape(KO2, P, HW).transpose(1, 0, 2)
        )

    nc = _get_nc()

    in_maps = []
    for c in range(NCORES):
        m = {"xt": np.ascontiguousarray(xt[c * PER : (c + 1) * PER])}
        for kind in "qkv":
            m[f"w{kind}1"] = w[f"w{kind}1"]
            m[f"w{kind}2"] = w[f"w{kind}2"]
        in_maps.append(m)

    from concourse.bass_utils import run_bass_kernel_spmd

    res = run_bass_kernel_spmd(
        nc,
        in_maps,
        core_ids=list(range(NCORES)),
        trace=_STATE.get("trace", False),
        **_STATE.get("run_kwargs", {}),
    )
    _STATE["last_result"] = res

    o = np.concatenate([r["o"] for r in res.results], axis=0)
    return o.reshape(N, C, H, W).astype(np.float32)



# revision 3
# speedup vs baseline: 1.2341x; 1.2341x over previous
"""Trainium2 Bass kernel for per-sample channel attention (fp8 DoubleRow).

Reference computation (per sample n of 32):
    e  = x[n].reshape(C, HW)                      # C=512, HW=1024
    q  = sigmoid(relu(e @ wq1) @ wq2)             # [C, HW]
    k  = sigmoid(relu(e @ wk1) @ wk2)             # [C, HW]
    v  = sigmoid(relu(e @ wv1) @ wv2)             # [C, HW]
    s  = q @ k.T / sqrt(C)                        # [C, C]
    o  = softmax(s, axis=-1) @ v                  # [C, HW]

Strategy: data-parallel over batch N across 8 cores (4 samples each),
weights replicated. All matmul operands are fp8 (e4m3) with
perf_mode=DoubleRow (2 fp8 weights per PE cell, 2 MACs/cycle — measured
~1.44x over bf16 at N=512); PSUM accumulates fp32.

Numerics tricks that make fp8 viable (absmax rel err ~5e-3 vs 2e-2 gate):
  - k,v are stored as t = tanh(z/2) in (-1,1) (sigmoid = (1+t)/2), q as
    sigma(z) in (0,1). Identities absorb all affine corrections:
      softmax_d(q·sigma_k) == softmax_d(S * 1/(2 sqrt C)),
        S[d,c] = sum_e t_k[d,e]·sigma_q[c,e]   (the q-rowsum term is
        constant per softmax row and cancels)
      P @ v = (P @ t_v + 1)/2                   (softmax rows sum to 1)
    Tanh+Exp+Sigmoid all live in ACT LUT tables such that only two
    table switches per sample occur, both prefetched into slack.
  - wv2 is quantized with error feedback along the contraction axis so
    each column's sum survives quantization; this kills the dominant
    error term (relu(h) has positive mean, so plain wv2 rounding noise
    is a common mode the softmax averaging cannot cancel).
  - w1,w2 are pre-scaled by 16 on host (fp8 subnormal avoidance); the
    1/256 folds into the ACT scale. Z uses a ones=2.0 vector so the
    reciprocal directly yields 0.5/Z, folding the (1+t)/2 un-mapping
    into the existing per-partition output scale.

Layouts (all SBUF-native, partition-major, pre-swizzled on host):
  eT   [P, KO1, C]  e^T chunks       qT [P, MT_E, C]  sigma_q^T
  kT   [P, MT_E, C] t_k^T            tv [P, MT_C, HW] t_v (natural)
  E    [P, MT_C, C] exp(scores^T)    softmax axis on partitions
DoubleRow contracts k-chunk PAIRS: lhsT [P, 2, M], rhs [P, 2, N=512].
"""

import math

import numpy as np
import ml_dtypes

N, C, H, W, R = 32, 512, 32, 32, 4
HW = H * W            # 1024
HID = HW // R         # 256
NCORES = 8
PER = N // NCORES     # samples per core
P = 128               # partitions

KO1 = HW // P         # 8  k-chunks for layer1
KP1 = KO1 // 2        # 4  DoubleRow pairs for layer1
KO2 = HID // P        # 2  k-chunks for layer2 (one DoubleRow pair)
MT_H = HID // P       # 2  m-tiles of h^T
MT_E = HW // P        # 8  m-tiles of q^T/k^T
KPE = MT_E // 2       # 4  DoubleRow pairs for scores contraction
MT_C = C // P         # 4  m-tiles over C
KPC = MT_C // 2       # 2  DoubleRow pairs for o contraction
NH = HW // C          # 2  halves of HW free dim (512 each)

S1 = 16.0             # host pre-scale on w1 (fp8 range use)
S2 = 16.0             # host pre-scale on w2

_STATE = {}

FP8 = ml_dtypes.float8_e4m3


def _build_nc():
    import concourse.bass as bass  # noqa: F401
    import concourse.mybir as mybir
    import concourse.tile as tile
    from concourse import bacc

    f8 = mybir.dt.float8e4
    f32 = mybir.dt.float32
    A = mybir.ActivationFunctionType
    DR = mybir.MatmulPerfMode.DoubleRow
    ALU = mybir.AluOpType

    nc = bacc.Bacc("TRN2")

    xt = nc.dram_tensor("xt", [PER, P, KO1, C], f8, kind="ExternalInput")
    ws1 = {
        kind: nc.dram_tensor(f"w{kind}1", [P, KO1, HID], f8, kind="ExternalInput")
        for kind in "qkv"
    }
    ws2 = {
        kind: nc.dram_tensor(f"w{kind}2", [P, KO2, HW], f8, kind="ExternalInput")
        for kind in "qkv"
    }
    out = nc.dram_tensor("o", [PER, C, HW], f32, kind="ExternalOutput")

    # exp scale: logits_eff = S * 1/(2 sqrt C)  (see module docstring)
    exp_scale = 1.0 / (2.0 * math.sqrt(C))
    sig_scale = 1.0 / (S1 * S2)        # sigma(z2) from psum z2' = S1*S2*z2
    tanh_scale = 1.0 / (2.0 * S1 * S2)  # tanh(z2/2)

    with tile.TileContext(nc) as tc:
        with (
            tc.tile_pool(name="singles", bufs=1) as singles,
            tc.tile_pool(name="acts", bufs=2) as acts,
            tc.tile_pool(name="hts", bufs=3) as hts,
            tc.tile_pool(name="obuf", bufs=3) as obuf,
            tc.tile_pool(name="psum", bufs=3, space="PSUM") as psum,
            tc.tile_pool(name="psz", bufs=2, space="PSUM") as psz,
        ):
            w1_sb = {}
            w2_sb = {}
            for kind in "qkv":
                w1_sb[kind] = singles.tile(
                    [P, KO1, HID], f8, tag=f"w1{kind}", name=f"w1{kind}"
                )
                w2_sb[kind] = singles.tile(
                    [P, KO2, HW], f8, tag=f"w2{kind}", name=f"w2{kind}"
                )

            warm_sb = singles.tile([P, C], f8, tag="warm", name="warm")
            nc.gpsimd.memset(warm_sb, 0.0)

            # Input DMAs in need-order: wq1 + eT(s0) first (first matmuls),
            # the rest behind. HWDGE (sync) and SWDGE (gpsimd) run in
            # parallel; split the critical first wave across both.
            nc.sync.dma_start(out=w1_sb["q"], in_=ws1["q"][:])
            eT0 = acts.tile([P, KO1, C], f8, tag="eT", name="eT")
            nc.gpsimd.dma_start(out=eT0[:, 6:, :], in_=xt[0][:, 6:, :])
            nc.sync.dma_start(out=eT0[:, :2, :], in_=xt[0][:, :2, :])
            nc.sync.dma_start(out=eT0[:, 2:4, :], in_=xt[0][:, 2:4, :])
            nc.sync.dma_start(out=eT0[:, 4:6, :], in_=xt[0][:, 4:6, :])
            nc.sync.dma_start(out=w2_sb["q"], in_=ws2["q"][:])
            nc.sync.dma_start(out=w1_sb["k"], in_=ws1["k"][:])
            nc.sync.dma_start(out=w2_sb["k"], in_=ws2["k"][:])
            nc.sync.dma_start(out=w1_sb["v"], in_=ws1["v"][:])
            nc.sync.dma_start(out=w2_sb["v"], in_=ws2["v"][:])

            # ones = 2.0 so the Z reciprocal yields 0.5/Z directly
            ones_sb = singles.tile([P, 1], f8, tag="ones", name="ones")
            nc.vector.memset(ones_sb, 2.0)
            actw = singles.tile([P, 1], f32, tag="actw", name="actw")

            # PE clock warm-up (HAM un-throttles after ~3.4us of activity)
            warm_ps = psum.tile([P, 2, C], f32, tag="ps", name="ps")
            for _ in range(10):
                nc.tensor.matmul(
                    warm_ps[:, 0, :], warm_sb[:, :P], warm_sb, start=True, stop=True
                )
            # Preload the sigmoid table set (contains sigmoid+tanh) while
            # DMAs land.
            nc.scalar.activation(actw, warm_sb[:, :1], A.Sigmoid)

            for s in range(PER):
                if s == 0:
                    eT = eT0
                else:
                    eT = acts.tile([P, KO1, C], f8, tag="eT", name="eT")
                    nc.scalar.dma_start(out=eT, in_=xt[s])

                qT = acts.tile([P, MT_E, C], f8, tag="qT", name="qT")
                kT = acts.tile([P, MT_E, C], f8, tag="kT", name="kT")
                tv = acts.tile([P, MT_C, HW], f8, tag="tv", name="tv")

                # sample 0's eT pairs arrive sync(0,1),(2,3),(4,5) +
                # gpsimd(6,7); accumulate in arrival order (order is free)
                kporder = (0, 1, 3, 2) if s == 0 else tuple(range(KP1))
                for kind in "qkv":
                    # layer 1: h^T[r, c], DoubleRow over k-chunk pairs
                    ps1 = psum.tile([P, MT_H, C], f32, tag="ps", name="ps")
                    for m in range(MT_H):
                        for kpi, kp in enumerate(kporder):
                            nc.tensor.matmul(
                                ps1[:, m, :],
                                w1_sb[kind][:, 2 * kp : 2 * kp + 2, m * P : (m + 1) * P],
                                eT[:, 2 * kp : 2 * kp + 2, :],
                                start=(kpi == 0),
                                stop=(kpi == KP1 - 1),
                                perf_mode=DR,
                            )
                    # relu PSUM -> fp8 SBUF (both m-tiles in one DVE op)
                    hT = hts.tile([P, KO2, C], f8, tag="hT", name="hT")
                    nc.vector.tensor_scalar_max(hT[:], ps1[:], 0.0)

                    if kind in ("q", "k"):
                        dst = qT if kind == "q" else kT
                        fn = A.Sigmoid if kind == "q" else A.Tanh
                        sc = sig_scale if kind == "q" else tanh_scale
                        # layer 2 transposed: z2^T[e, c]; one DoubleRow per
                        # m-tile; ACT over psum pairs (2 banks) amortizes
                        # the per-instruction bubble.
                        for mp in range(MT_E // 2):
                            ps2 = psum.tile([P, 2, C], f32, tag="ps", name="ps")
                            for mi in range(2):
                                m = 2 * mp + mi
                                nc.tensor.matmul(
                                    ps2[:, mi, :],
                                    w2_sb[kind][:, 0:2, m * P : (m + 1) * P],
                                    hT[:, 0:2, :],
                                    start=True,
                                    stop=True,
                                    perf_mode=DR,
                                )
                            nc.scalar.activation(
                                dst[:, 2 * mp : 2 * mp + 2, :], ps2[:], fn, scale=sc
                            )
                    else:
                        # v natural: z2[d, e] = (h^T)^T @ w2
                        for m in range(MT_C):
                            ps2 = psum.tile([P, 2, C], f32, tag="ps", name="ps")
                            for nh in range(NH):
                                nc.tensor.matmul(
                                    ps2[:, nh, :],
                                    hT[:, 0:2, m * P : (m + 1) * P],
                                    w2_sb[kind][:, 0:2, nh * C : (nh + 1) * C],
                                    start=True,
                                    stop=True,
                                    perf_mode=DR,
                                )
                            nc.scalar.activation(
                                tv[:, m, :], ps2[:], A.Tanh, scale=tanh_scale
                            )
                # prefetch the exp table into the scores-matmul window
                # (depends on the last tanh output so it lands after it)
                nc.scalar.activation(actw, tv[:, MT_C - 1, HW - 1 :], A.Exp)

                # scores transposed: S^T[d, c] = sum_e t_k[d,e] sigma_q[c,e],
                # E = exp(S^T * exp_scale) in fp8
                E = acts.tile([P, MT_C, C], f8, tag="E", name="E")
                for mp in range(MT_C // 2):
                    ps = psum.tile([P, 2, C], f32, tag="ps", name="ps")
                    for mi in range(2):
                        m = 2 * mp + mi
                        for kp in range(KPE):
                            nc.tensor.matmul(
                                ps[:, mi, :],
                                kT[:, 2 * kp : 2 * kp + 2, m * P : (m + 1) * P],
                                qT[:, 2 * kp : 2 * kp + 2, :],
                                start=(kp == 0),
                                stop=(kp == KPE - 1),
                                perf_mode=DR,
                            )
                    nc.scalar.activation(
                        E[:, 2 * mp : 2 * mp + 2, :], ps[:], A.Exp, scale=exp_scale
                    )
                if s < PER - 1:
                    # pull the sigmoid-table reload into the o-matmul window
                    nc.scalar.activation(actw, E[:, MT_C - 1, :1], A.Sigmoid)

                # softmax denominator: rz[c] = 0.5 / Z[c]  (ones = 2.0)
                rz = obuf.tile([P, MT_C], f32, tag="rz", name="rz")
                for m in range(MT_C):
                    pz = psz.tile([P, 1], f32, tag="pz", name="pz")
                    for k in range(MT_C):
                        nc.tensor.matmul(
                            pz,
                            E[:, k, m * P : (m + 1) * P],
                            ones_sb,
                            start=(k == 0),
                            stop=(k == MT_C - 1),
                        )
                    nc.vector.reciprocal(rz[:, m : m + 1], pz)

                # o[c, e] = (E^T @ t_v) * rz[c] + 0.5
                out_r = out[s].rearrange("(mo p) e -> p mo e", p=P)
                for m in range(MT_C):
                    ob = obuf.tile([P, HW], f32, tag="ob", name="ob")
                    ps = psum.tile([P, 2, C], f32, tag="ps", name="ps")
                    for kp in range(KPC):
                        for nh in range(NH):
                            nc.tensor.matmul(
                                ps[:, nh, :],
                                E[:, 2 * kp : 2 * kp + 2, m * P : (m + 1) * P],
                                tv[:, 2 * kp : 2 * kp + 2, nh * C : (nh + 1) * C],
                                start=(kp == 0),
                                stop=(kp == KPC - 1),
                                perf_mode=DR,
                            )
                    if s < PER - 1:
                        nc.vector.tensor_scalar(
                            ob[:], ps[:], rz[:, m : m + 1], 0.5, ALU.mult, ALU.add
                        )
                        eng = (nc.sync, nc.scalar)[m % 2]
                        eng.dma_start(out=out_r[:, m, :], in_=ob[:])
                    else:
                        # final sample: split scale+store into halves so the
                        # tail chain after the last matmul is shorter
                        for nh in range(NH):
                            nc.vector.tensor_scalar(
                                ob[:, nh * C : (nh + 1) * C],
                                ps[:, nh, :],
                                rz[:, m : m + 1],
                                0.5,
                                ALU.mult,
                                ALU.add,
                            )
                            eng = (nc.sync, nc.scalar)[(m * NH + nh) % 2]
                            eng.dma_start(
                                out=out_r[:, m, nh * C : (nh + 1) * C],
                                in_=ob[:, nh * C : (nh + 1) * C],
                            )

    nc.finalize()
    return nc


def _get_nc():
    if "nc" not in _STATE:
        _STATE["nc"] = _build_nc()
    return _STATE["nc"]


def _quant_ef(a):
    """fp8 quantization with error feedback along axis 0 (contraction axis):
    carry the rounding residual so each column's sum is preserved."""
    out = np.empty(a.shape, dtype=FP8)
    c = np.zeros(a.shape[1:], dtype=np.float32)
    for h in range(a.shape[0]):
        u = a[h] + c
        q = u.astype(FP8)
        c = u - q.astype(np.float32)
        out[h] = q
    return out


def kernel(**inputs):
    x = np.asarray(inputs["x"])

    # host-side reformat to SBUF-native layouts (+ fp8 cast):
    #   x:  [N, C, H, W] -> e^T [N, HW, C] -> [N, P, KO1, C]
    #   w1: [HW, HID] * 16 -> [P, KO1, HID]
    #   w2: [HID, HW] * 16 -> [P, KO2, HW]  (wv2 with error feedback)
    xt = np.ascontiguousarray(
        x.reshape(N, C, HW)
        .transpose(0, 2, 1)
        .reshape(N, KO1, P, C)
        .transpose(0, 2, 1, 3)
    ).astype(FP8)
    w = {}
    for name in ("wq1", "wk1", "wv1"):
        a = (np.asarray(inputs[name]) * S1).astype(FP8)
        w[name] = np.ascontiguousarray(a.reshape(KO1, P, HID).transpose(1, 0, 2))
    for name in ("wq2", "wk2", "wv2"):
        a = np.asarray(inputs[name]).astype(np.float32) * S2
        a8 = _quant_ef(a) if name == "wv2" else a.astype(FP8)
        w[name] = np.ascontiguousarray(a8.reshape(KO2, P, HW).transpose(1, 0, 2))

    nc = _get_nc()

    in_maps = []
    for c in range(NCORES):
        m = {"xt": np.ascontiguousarray(xt[c * PER : (c + 1) * PER])}
        for kind in "qkv":
            m[f"w{kind}1"] = w[f"w{kind}1"]
            m[f"w{kind}2"] = w[f"w{kind}2"]
        in_maps.append(m)

    from concourse.bass_utils import run_bass_kernel_spmd

    res = run_bass_kernel_spmd(
        nc,
        in_maps,
        core_ids=list(range(NCORES)),
        trace=_STATE.get("trace", False),
        **_STATE.get("run_kwargs", {}),
    )
    _STATE["last_result"] = res

    o = np.concatenate([r["o"] for r in res.results], axis=0)
    return o.reshape(N, C, H, W).astype(np.float32)


# revision 4
# speedup vs baseline: 1.3813x; 1.1192x over previous
"""Trainium2 Bass kernel for per-sample channel attention (fp8 DoubleRow).

Reference computation (per sample n of 32):
    e  = x[n].reshape(C, HW)                      # C=512, HW=1024
    q  = sigmoid(relu(e @ wq1) @ wq2)             # [C, HW]
    k  = sigmoid(relu(e @ wk1) @ wk2)             # [C, HW]
    v  = sigmoid(relu(e @ wv1) @ wv2)             # [C, HW]
    s  = q @ k.T / sqrt(C)                        # [C, C]
    o  = softmax(s, axis=-1) @ v                  # [C, HW]

Strategy: data-parallel over batch N across 8 cores (4 samples each),
weights replicated. All matmul operands are fp8 (e4m3) with
perf_mode=DoubleRow (2 fp8 weights per PE cell, 2 MACs/cycle — measured
~1.44x over bf16 at N=512); PSUM accumulates fp32.

Numerics tricks that make fp8 viable (absmax rel err ~5e-3 vs 2e-2 gate):
  - k,v are stored as t = tanh(z/2) in (-1,1) (sigmoid = (1+t)/2), q as
    sigma(z) in (0,1). Identities absorb all affine corrections:
      softmax_d(q·sigma_k) == softmax_d(S * 1/(2 sqrt C)),
        S[d,c] = sum_e t_k[d,e]·sigma_q[c,e]   (the q-rowsum term is
        constant per softmax row and cancels)
      P @ v = (P @ t_v + 1)/2                   (softmax rows sum to 1)
    Tanh+Exp+Sigmoid all live in ACT LUT tables such that only two
    table switches per sample occur, both prefetched into slack.
  - wv2 is quantized with error feedback along the contraction axis so
    each column's sum survives quantization; this kills the dominant
    error term (relu(h) has positive mean, so plain wv2 rounding noise
    is a common mode the softmax averaging cannot cancel).
  - w1,w2 are pre-scaled by 16 on host (fp8 subnormal avoidance); the
    1/256 folds into the ACT scale. Z uses a ones=2.0 vector so the
    reciprocal directly yields 0.5/Z, folding the (1+t)/2 un-mapping
    into the existing per-partition output scale.

Layouts (all SBUF-native, partition-major, pre-swizzled on host):
  eT   [P, KO1, C]  e^T chunks       qT [P, MT_E, C]  sigma_q^T
  kT   [P, MT_E, C] t_k^T            tv [P, MT_C, HW] t_v (natural)
  E    [P, MT_C, C] exp(scores^T)    softmax axis on partitions
DoubleRow contracts k-chunk PAIRS: lhsT [P, 2, M], rhs [P, 2, N=512].
"""

import math

import numpy as np
import ml_dtypes

N, C, H, W, R = 32, 512, 32, 32, 4
HW = H * W            # 1024
HID = HW // R         # 256
NCORES = 8
PER = N // NCORES     # samples per core
P = 128               # partitions

KO1 = HW // P         # 8  k-chunks for layer1
KP1 = KO1 // 2        # 4  DoubleRow pairs for layer1
KO2 = HID // P        # 2  k-chunks for layer2 (one DoubleRow pair)
MT_H = HID // P       # 2  m-tiles of h^T
MT_E = HW // P        # 8  m-tiles of q^T/k^T
KPE = MT_E // 2       # 4  DoubleRow pairs for scores contraction
MT_C = C // P         # 4  m-tiles over C
KPC = MT_C // 2       # 2  DoubleRow pairs for o contraction
NH = HW // C          # 2  halves of HW free dim (512 each)

S1 = 16.0             # host pre-scale on w1 (fp8 range use)
S2 = 16.0             # host pre-scale on w2

_STATE = {}

FP8 = ml_dtypes.float8_e4m3


def _build_nc():
    import concourse.bass as bass  # noqa: F401
    import concourse.mybir as mybir
    import concourse.tile as tile
    from concourse import bacc

    f8 = mybir.dt.float8e4
    f32 = mybir.dt.float32
    A = mybir.ActivationFunctionType
    DR = mybir.MatmulPerfMode.DoubleRow
    ALU = mybir.AluOpType

    nc = bacc.Bacc("TRN2")

    xt = nc.dram_tensor("xt", [PER, P, KO1, C], f8, kind="ExternalInput")
    ws1 = {
        kind: nc.dram_tensor(f"w{kind}1", [P, KO1, HID], f8, kind="ExternalInput")
        for kind in "qkv"
    }
    ws2 = {
        kind: nc.dram_tensor(f"w{kind}2", [P, KO2, HW], f8, kind="ExternalInput")
        for kind in "qkv"
    }
    out = nc.dram_tensor("o", [PER, C, HW], f32, kind="ExternalOutput")

    # exp scale: logits_eff = S * 1/(2 sqrt C)  (see module docstring)
    exp_scale = 1.0 / (2.0 * math.sqrt(C))
    sig_scale = 1.0 / (S1 * S2)        # sigma(z2) from psum z2' = S1*S2*z2
    tanh_scale = 1.0 / (2.0 * S1 * S2)  # tanh(z2/2)

    with tile.TileContext(nc) as tc:
        with (
            tc.tile_pool(name="singles", bufs=1) as singles,
            tc.tile_pool(name="acts", bufs=2) as acts,
            tc.tile_pool(name="hts", bufs=3) as hts,
            tc.tile_pool(name="obuf", bufs=3) as obuf,
            tc.tile_pool(name="psum", bufs=3, space="PSUM") as psum,
            tc.tile_pool(name="psz", bufs=2, space="PSUM") as psz,
        ):
            w1_sb = {}
            w2_sb = {}
            for kind in "qkv":
                w1_sb[kind] = singles.tile(
                    [P, KO1, HID], f8, tag=f"w1{kind}", name=f"w1{kind}"
                )
                w2_sb[kind] = singles.tile(
                    [P, KO2, HW], f8, tag=f"w2{kind}", name=f"w2{kind}"
                )

            warm_sb = singles.tile([P, C], f8, tag="warm", name="warm")
            nc.gpsimd.memset(warm_sb, 0.0)

            # Input DMAs in need-order: wq1 + eT(s0) first (first matmuls),
            # the rest behind. HWDGE (sync) and SWDGE (gpsimd) run in
            # parallel; split the critical first wave across both.
            nc.sync.dma_start(out=w1_sb["q"], in_=ws1["q"][:])
            eT0 = acts.tile([P, KO1, C], f8, tag="eT", name="eT")
            nc.gpsimd.dma_start(out=eT0[:, 6:, :], in_=xt[0][:, 6:, :])
            nc.sync.dma_start(out=eT0[:, :2, :], in_=xt[0][:, :2, :])
            nc.sync.dma_start(out=eT0[:, 2:4, :], in_=xt[0][:, 2:4, :])
            nc.sync.dma_start(out=eT0[:, 4:6, :], in_=xt[0][:, 4:6, :])
            nc.sync.dma_start(out=w2_sb["q"], in_=ws2["q"][:])
            nc.sync.dma_start(out=w1_sb["k"], in_=ws1["k"][:])
            nc.sync.dma_start(out=w2_sb["k"], in_=ws2["k"][:])
            nc.sync.dma_start(out=w1_sb["v"], in_=ws1["v"][:])
            nc.sync.dma_start(out=w2_sb["v"], in_=ws2["v"][:])

            # ones = 2.0 so the Z reciprocal yields 0.5/Z directly
            ones_sb = singles.tile([P, 1], f8, tag="ones", name="ones")
            nc.vector.memset(ones_sb, 2.0)
            actw = singles.tile([P, 1], f32, tag="actw", name="actw")

            # PE clock warm-up (HAM un-throttles after ~3.4us of activity)
            warm_ps = psum.tile([P, 2, C], f32, tag="ps", name="ps")
            for _ in range(7):
                nc.tensor.matmul(
                    warm_ps[:, 0, :], warm_sb[:, :P], warm_sb, start=True, stop=True
                )
            # Preload the sigmoid table set (contains sigmoid+tanh) while
            # DMAs land.
            nc.scalar.activation(actw, warm_sb[:, :1], A.Sigmoid)

            def layer1(kind, eT, kporder):
                # layer 1: h^T[r, c], DoubleRow over k-chunk pairs, then
                # relu PSUM -> fp8 SBUF (both m-tiles in one DVE op). The
                # relu latency is hidden under the NEXT block's matmuls.
                ps1 = psum.tile([P, MT_H, C], f32, tag="ps", name="ps")
                for m in range(MT_H):
                    for kpi, kp in enumerate(kporder):
                        nc.tensor.matmul(
                            ps1[:, m, :],
                            w1_sb[kind][:, 2 * kp : 2 * kp + 2, m * P : (m + 1) * P],
                            eT[:, 2 * kp : 2 * kp + 2, :],
                            start=(kpi == 0),
                            stop=(kpi == KP1 - 1),
                            perf_mode=DR,
                        )
                hT = hts.tile([P, KO2, C], f8, tag="hT", name="hT")
                nc.vector.tensor_scalar_max(hT[:], ps1[:], 0.0)
                return hT

            def layer2_qk(kind, hT, dst):
                fn = A.Sigmoid if kind == "q" else A.Tanh
                sc = sig_scale if kind == "q" else tanh_scale
                # layer 2 transposed: z2^T[e, c]; one DoubleRow per m-tile;
                # ACT over psum pairs (2 banks) amortizes the bubble.
                for mp in range(MT_E // 2):
                    ps2 = psum.tile([P, 2, C], f32, tag="ps", name="ps")
                    for mi in range(2):
                        m = 2 * mp + mi
                        nc.tensor.matmul(
                            ps2[:, mi, :],
                            w2_sb[kind][:, 0:2, m * P : (m + 1) * P],
                            hT[:, 0:2, :],
                            start=True,
                            stop=True,
                            perf_mode=DR,
                        )
                    nc.scalar.activation(
                        dst[:, 2 * mp : 2 * mp + 2, :], ps2[:], fn, scale=sc
                    )

            def layer2_v(hT, tv):
                # v natural: z2[d, e] = (h^T)^T @ w2
                for m in range(MT_C):
                    ps2 = psum.tile([P, 2, C], f32, tag="ps", name="ps")
                    for nh in range(NH):
                        nc.tensor.matmul(
                            ps2[:, nh, :],
                            hT[:, 0:2, m * P : (m + 1) * P],
                            w2_sb["v"][:, 0:2, nh * C : (nh + 1) * C],
                            start=True,
                            stop=True,
                            perf_mode=DR,
                        )
                    nc.scalar.activation(tv[:, m, :], ps2[:], A.Tanh, scale=tanh_scale)

            def scores(qT, kT, E):
                # S^T[d, c] = sum_e t_k[d,e] sigma_q[c,e]; E = exp fp8
                for mp in range(MT_C // 2):
                    ps = psum.tile([P, 2, C], f32, tag="ps", name="ps")
                    for mi in range(2):
                        m = 2 * mp + mi
                        for kp in range(KPE):
                            nc.tensor.matmul(
                                ps[:, mi, :],
                                kT[:, 2 * kp : 2 * kp + 2, m * P : (m + 1) * P],
                                qT[:, 2 * kp : 2 * kp + 2, :],
                                start=(kp == 0),
                                stop=(kp == KPE - 1),
                                perf_mode=DR,
                            )
                    nc.scalar.activation(
                        E[:, 2 * mp : 2 * mp + 2, :], ps[:], A.Exp, scale=exp_scale
                    )

            def z_and_o(s, E, tv):
                # softmax denominator rz[c] = 0.5/Z[c] (ones = 2.0), then
                # o[c, e] = (E^T @ t_v) * rz[c] + 0.5
                rz = obuf.tile([P, MT_C], f32, tag="rz", name="rz")
                for m in range(MT_C):
                    pz = psz.tile([P, 1], f32, tag="pz", name="pz")
                    for k in range(MT_C):
                        nc.tensor.matmul(
                            pz,
                            E[:, k, m * P : (m + 1) * P],
                            ones_sb,
                            start=(k == 0),
                            stop=(k == MT_C - 1),
                        )
                    nc.vector.reciprocal(rz[:, m : m + 1], pz)

                out_r = out[s].rearrange("(mo p) e -> p mo e", p=P)
                for m in range(MT_C):
                    ob = obuf.tile([P, HW], f32, tag="ob", name="ob")
                    ps = psum.tile([P, 2, C], f32, tag="ps", name="ps")
                    for kp in range(KPC):
                        for nh in range(NH):
                            nc.tensor.matmul(
                                ps[:, nh, :],
                                E[:, 2 * kp : 2 * kp + 2, m * P : (m + 1) * P],
                                tv[:, 2 * kp : 2 * kp + 2, nh * C : (nh + 1) * C],
                                start=(kp == 0),
                                stop=(kp == KPC - 1),
                                perf_mode=DR,
                            )
                    if s < PER - 1:
                        nc.vector.tensor_scalar(
                            ob[:], ps[:], rz[:, m : m + 1], 0.5, ALU.mult, ALU.add
                        )
                        eng = (nc.sync, nc.scalar)[m % 2]
                        eng.dma_start(out=out_r[:, m, :], in_=ob[:])
                    else:
                        # final sample: split scale+store into halves so the
                        # tail chain after the last matmul is shorter
                        for nh in range(NH):
                            nc.vector.tensor_scalar(
                                ob[:, nh * C : (nh + 1) * C],
                                ps[:, nh, :],
                                rz[:, m : m + 1],
                                0.5,
                                ALU.mult,
                                ALU.add,
                            )
                            eng = (nc.sync, nc.scalar)[(m * NH + nh) % 2]
                            eng.dma_start(
                                out=out_r[:, m, nh * C : (nh + 1) * C],
                                in_=ob[:, nh * C : (nh + 1) * C],
                            )

            # Software pipeline: within a sample the three projections are
            # interleaved (each relu hides under the next block's matmuls);
            # across samples, Z+output of sample s-1 run between sample s's
            # projections and scores, so exp/reciprocal latency never stalls
            # the PE.
            prev = None
            for s in range(PER):
                if s == 0:
                    eT = eT0
                else:
                    eT = acts.tile([P, KO1, C], f8, tag="eT", name="eT")
                    nc.scalar.dma_start(out=eT, in_=xt[s])

                qT = acts.tile([P, MT_E, C], f8, tag="qT", name="qT")
                kT = acts.tile([P, MT_E, C], f8, tag="kT", name="kT")
                tv = acts.tile([P, MT_C, HW], f8, tag="tv", name="tv")

                # sample 0's eT pairs arrive sync(0,1),(2,3),(4,5) +
                # gpsimd(6,7); accumulate in arrival order (order is free)
                kporder = (0, 1, 3, 2) if s == 0 else tuple(range(KP1))
                hq = layer1("q", eT, kporder)
                hk = layer1("k", eT, kporder)
                layer2_qk("q", hq, qT)
                hv = layer1("v", eT, kporder)
                layer2_qk("k", hk, kT)
                layer2_v(hv, tv)
                # prefetch the exp table (depends on the last tanh output)
                nc.scalar.activation(actw, tv[:, MT_C - 1, HW - 1 :], A.Exp)

                if prev is not None:
                    z_and_o(*prev)

                E = acts.tile([P, MT_C, C], f8, tag="E", name="E")
                scores(qT, kT, E)
                if s < PER - 1:
                    # pull the sigmoid-table reload into the next window
                    nc.scalar.activation(actw, E[:, MT_C - 1, :1], A.Sigmoid)
                prev = (s, E, tv)

            z_and_o(*prev)

    nc.finalize()
    return nc


def _get_nc():
    if "nc" not in _STATE:
        _STATE["nc"] = _build_nc()
    return _STATE["nc"]


def _quant_ef(a):
    """fp8 quantization with error feedback along axis 0 (contraction axis):
    carry the rounding residual so each column's sum is preserved."""
    out = np.empty(a.shape, dtype=FP8)
    c = np.zeros(a.shape[1:], dtype=np.float32)
    for h in range(a.shape[0]):
        u = a[h] + c
        q = u.astype(FP8)
        c = u - q.astype(np.float32)
        out[h] = q
    return out


def kernel(**inputs):
    x = np.asarray(inputs["x"])

    # host-side reformat to SBUF-native layouts (+ fp8 cast):
    #   x:  [N, C, H, W] -> e^T [N, HW, C] -> [N, P, KO1, C]
    #   w1: [HW, HID] * 16 -> [P, KO1, HID]
    #   w2: [HID, HW] * 16 -> [P, KO2, HW]  (wv2 with error feedback)
    xt = np.ascontiguousarray(
        x.reshape(N, C, HW)
        .transpose(0, 2, 1)
        .reshape(N, KO1, P, C)
        .transpose(0, 2, 1, 3)
    ).astype(FP8)
    w = {}
    for name in ("wq1", "wk1", "wv1"):
        a = (np.asarray(inputs[name]) * S1).astype(FP8)
        w[name] = np.ascontiguousarray(a.reshape(KO1, P, HID).transpose(1, 0, 2))
    for name in ("wq2", "wk2", "wv2"):
        a = np.asarray(inputs[name]).astype(np.float32) * S2
        a8 = _quant_ef(a) if name == "wv2" else a.astype(FP8)
        w[name] = np.ascontiguousarray(a8.reshape(KO2, P, HW).transpose(1, 0, 2))

    nc = _get_nc()

    in_maps = []
    for c in range(NCORES):
        m = {"xt": np.ascontiguousarray(xt[c * PER : (c + 1) * PER])}
        for kind in "qkv":
            m[f"w{kind}1"] = w[f"w{kind}1"]
            m[f"w{kind}2"] = w[f"w{kind}2"]
        in_maps.append(m)

    from concourse.bass_utils import run_bass_kernel_spmd

    res = run_bass_kernel_spmd(
        nc,
        in_maps,
        core_ids=list(range(NCORES)),
        trace=_STATE.get("trace", False),
        **_STATE.get("run_kwargs", {}),
    )
    _STATE["last_result"] = res

    o = np.concatenate([r["o"] for r in res.results], axis=0)
    return o.reshape(N, C, H, W).astype(np.float32)
